# revision 11
# baseline (speedup 1.0000x reference)
"""Trainium2 Bass kernel for nn_CHARM_89146341196444 (gnn_message_passing).

Reference (N=8192, FEAT=1024, HID=512, DK=256, KMAX=8):
    dense = relu(X @ W1 + b1); q = dense @ Wq / 16; k = dense @ Wk
    v = dense @ Wv + bv
    a_kk[i] = sum_{j<=kk} q[i] . k[sp[i, j]]          (kk in {2,4,8})
    alpha_kk = softmax(a_kk) over all N rows
    out = softmax((sum_kk sum_i alpha_kk[i] v[i]) @ Wout + bout)   # [1, 2]

The [N, N] attention matrix is never materialized: only <=9 entries per row
are used, and each is a q.k dot — so we gather the needed k rows (GPSIMD
ap_gather from an SBUF table, no DMA descriptors) and reduce.

Distribution: rows sharded 1024/core on 8 cores. k is computed in a packed
transposed layout kT2[p, i, e] = k[i, e*128+p] (f16), AllGather'ed (4 MB),
and gathered per (row, neighbor). Per-core partial sums P_kk (=sum e_i v_i)
and Z_kk (=sum e_i) with a constant exp-shift (cancels in P/Z) return to the
host, which finishes the tiny [256] -> [1, 2] head.
"""

import numpy as np

P = 128
NL = 1024          # rows per core
FEAT = 1024
HID = 512
DK = 256
KMAX1 = 9
NCORES = 8
N = NL * NCORES
IC = NL // P       # 8
FC = FEAT // P     # 8
HC = HID // P      # 4
NI = NL * KMAX1    # 9216 gather indices per core
EXP_SHIFT = 12.0
IT = 2             # fc1 i-tiles (512 wide)
ITW = NL // IT


def build_body(tc, t):
    import concourse.mybir as mybir

    nc = tc.nc
    f16 = mybir.dt.float16
    f32 = mybir.dt.float32
    Relu = mybir.ActivationFunctionType.Relu
    Exp = mybir.ActivationFunctionType.Exp
    X = mybir.AxisListType.X
    mult = mybir.AluOpType.mult
    add = mybir.AluOpType.add
    bypass = mybir.AluOpType.bypass

    with (
        tc.tile_pool(name="wp", bufs=1) as wp,
        tc.tile_pool(name="per", bufs=1) as per,
        tc.tile_pool(name="pp", bufs=2, space="PSUM") as pp,
    ):
        # ---- weights / constants ----
        w1_sb = wp.tile([P, FC, HID], f16)
        nc.sync.dma_start(out=w1_sb[:], in_=t["w1"])
        wq_sb = wp.tile([P, HC, 2, P], f16)
        nc.sync.dma_start(out=wq_sb[:], in_=t["wq"])
        wk_sb = wp.tile([P, HC, 2, P], f16)
        nc.sync.dma_start(out=wk_sb[:], in_=t["wk"])
        wv_sb = wp.tile([P, HC, 2, P], f16)
        nc.sync.dma_start(out=wv_sb[:], in_=t["wv"])
        b1_sb = wp.tile([P, HC], f32)
        nc.sync.dma_start(out=b1_sb[:], in_=t["b1v"])
        idx_sb = wp.tile([P, NI // 16], mybir.dt.int16)
        nc.sync.dma_start(out=idx_sb[:], in_=t["idx"])
        ones_sb = wp.tile([P, P], f16)
        nc.vector.memset(ones_sb[:], 1.0)
        shift_sb = wp.tile([P, 1], f32)
        nc.vector.memset(shift_sb[:], -EXP_SHIFT)

        # ---- persistent activations ----
        denseT = per.tile([P, HC, NL], f16)       # [h, i]
        q2 = per.tile([P, NL, 2], f16)            # q[i, e*128+p] interleaved
        k2 = per.tile([P, NL, 2], f16)            # same for k
        v_cm = per.tile([P, 2, NL], f16)          # v chunk-major
        s2 = per.tile([P, NL, 2], f16)
        s4 = per.tile([P, NL, 2], f16)
        s8 = per.tile([P, NL, 2], f16)
        e_bc = per.tile([P, 3, NL], f16)          # exp(a - shift), bcast rows
        p_sb = per.tile([P, 2, 4], f32)
        z_sb = per.tile([P, 4], f32)

        # ---- fc1: denseT = relu(W1^T X^T + b1) ----
        with tc.tile_pool(name="xf", bufs=2) as xf:
            for it in range(IT):
                sl = slice(it * ITW, (it + 1) * ITW)
                xt_f = xf.tile([P, FC, ITW], f32, tag="xtf")
                nc.sync.dma_start(out=xt_f[:], in_=t["xt"][:, :, sl])
                xtb = xf.tile([P, FC, ITW], f16, tag="xtb")
                nc.scalar.copy(out=xtb[:], in_=xt_f[:])
                for hc in range(HC):
                    ps = pp.tile([P, ITW], f32, tag="ps")
                    for fc in range(FC):
                        nc.tensor.matmul(
                            out=ps[:],
                            lhsT=w1_sb[:, fc, hc * P:(hc + 1) * P],
                            rhs=xtb[:, fc, :],
                            start=(fc == 0),
                            stop=(fc == FC - 1),
                        )
                    nc.scalar.activation(
                        out=denseT[:, hc, sl], in_=ps[:],
                        func=Relu, bias=b1_sb[:, hc:hc + 1],
                    )

            # ---- kT/qT/vT chunks: lhsT = W*[:, hc, e, :], rhs = denseT ----
            def proj(w_sb, writeback):
                for e in range(2):
                    for it in range(IT):
                        sl = slice(it * ITW, (it + 1) * ITW)
                        ps = pp.tile([P, ITW], f32, tag="ps")
                        for hc in range(HC):
                            nc.tensor.matmul(
                                out=ps[:],
                                lhsT=w_sb[:, hc, e, :],
                                rhs=denseT[:, hc, sl],
                                start=(hc == 0),
                                stop=(hc == HC - 1),
                            )
                        writeback(ps, e, sl)

            # k first: it feeds the AllGather
            proj(wk_sb, lambda ps, e, sl: nc.scalar.copy(
                out=k2[:, sl, e], in_=ps[:]))
            nc.sync.dma_start(out=t["k_cc"], in_=k2[:])
            nc.gpsimd.collective_compute(
                "AllGather",
                bypass,
                replica_groups=[list(range(NCORES))],
                ins=[t["k_cc"].opt()],
                outs=[t["k_all"].opt()],
            )
            proj(wq_sb, lambda ps, e, sl: nc.scalar.copy(
                out=q2[:, sl, e], in_=ps[:]))
            proj(wv_sb, lambda ps, e, sl: nc.vector.tensor_copy(
                out=v_cm[:, e, sl], in_=ps[:]))

        # ---- gather table + ap_gather ----
        with (
            tc.tile_pool(name="gp", bufs=1) as gp,
            tc.tile_pool(name="sp", bufs=2) as sp,
        ):
            tab = gp.tile([P, NCORES, NL, 2], f16)
            nc.sync.dma_start(
                out=tab[:],
                in_=t["k_all"].rearrange("(c p) i e -> p c i e", p=P),
            )
            g = gp.tile([P, NI, 2], f16)
            nc.gpsimd.ap_gather(
                out_ap=g[:],
                in_ap=tab.rearrange("p c i e -> p (c i) e")[:],
                idxs_ap=idx_sb[:],
                channels=P, num_elems=N, d=2, num_idxs=NI,
            )
            # t = j*NL + i  ->  g viewed [P, 9, NL, 2]
            gv = g.rearrange("p (j i) e -> p j i e", j=KMAX1)
            nc.vector.tensor_tensor(out=s2[:], in0=gv[:, 0], in1=gv[:, 1], op=add)
            nc.vector.tensor_tensor(out=s2[:], in0=s2[:], in1=gv[:, 2], op=add)
            nc.vector.tensor_tensor(out=s4[:], in0=gv[:, 3], in1=gv[:, 4], op=add)
            nc.vector.tensor_tensor(out=s4[:], in0=s4[:], in1=s2[:], op=add)
            nc.vector.tensor_tensor(out=s8[:], in0=gv[:, 5], in1=gv[:, 6], op=add)
            nc.vector.tensor_tensor(out=s8[:], in0=s8[:], in1=gv[:, 7], op=add)
            nc.vector.tensor_tensor(out=s8[:], in0=s8[:], in1=gv[:, 8], op=add)
            nc.vector.tensor_tensor(out=s8[:], in0=s8[:], in1=s4[:], op=add)

            # ---- a_kk = q . s_kk; e_kk = exp(a_kk - shift), broadcast ----
            for kk, s in enumerate((s2, s4, s8)):
                aq = sp.tile([P, NL, 2], f16, tag="aq")
                nc.vector.tensor_tensor(out=aq[:], in0=q2[:], in1=s[:], op=mult)
                at = sp.tile([P, NL], f16, tag="at")
                with nc.allow_low_precision(
                        reason="2-way f16 add; PE accumulates the 256-dot in f32"):
                    nc.vector.reduce_sum(out=at[:], in_=aq[:], axis=X)
                for half in range(2):
                    hs = slice(half * ITW, (half + 1) * ITW)
                    psa = pp.tile([P, ITW], f32, tag="psa")
                    nc.tensor.matmul(
                        out=psa[:], lhsT=ones_sb[:], rhs=at[:, hs],
                        start=True, stop=True,
                    )
                    nc.scalar.activation(
                        out=e_bc[:, kk, hs], in_=psa[:],
                        func=Exp, bias=shift_sb[:],
                    )

            # ---- P_kk, Z_kk partials ----
            nc.vector.reduce_sum(out=z_sb[:, :3], in_=e_bc[:], axis=X)
            for kk in range(3):
                ev = sp.tile([P, 2, NL], f16, tag="ev")
                nc.vector.tensor_tensor(
                    out=ev[:],
                    in0=v_cm[:],
                    in1=e_bc[:, kk:kk + 1, :].to_broadcast((P, 2, NL)),
                    op=mult,
                )
                nc.vector.reduce_sum(out=p_sb[:, :, kk], in_=ev[:], axis=X)

            nc.sync.dma_start(out=t["out_p"], in_=p_sb[:, :, :3])
            nc.sync.dma_start(out=t["out_z"], in_=z_sb[:1, :3])


def build_nc():
    import concourse.mybir as mybir
    import concourse.tile as tile
    from concourse import bacc

    f16 = mybir.dt.float16
    f32 = mybir.dt.float32

    nc = bacc.Bacc("TRN2", target_bir_lowering=False, debug=False,
                   num_devices=NCORES)
    t = {}
    t["xt"] = nc.declare_dram_parameter("xt", [P, FC, NL], f32, isOutput=False).ap()
    t["w1"] = nc.declare_dram_parameter("w1", [P, FC, HID], f16, isOutput=False).ap()
    t["wq"] = nc.declare_dram_parameter("wq", [P, HC, 2, P], f16, isOutput=False).ap()
    t["wk"] = nc.declare_dram_parameter("wk", [P, HC, 2, P], f16, isOutput=False).ap()
    t["wv"] = nc.declare_dram_parameter("wv", [P, HC, 2, P], f16, isOutput=False).ap()
    t["b1v"] = nc.declare_dram_parameter("b1v", [P, HC], f32, isOutput=False).ap()
    t["idx"] = nc.declare_dram_parameter(
        "idx", [P, NI // 16], mybir.dt.int16, isOutput=False).ap()
    t["out_p"] = nc.declare_dram_parameter("out_p", [P, 2, 3], f32, isOutput=True).ap()
    t["out_z"] = nc.declare_dram_parameter("out_z", [1, 3], f32, isOutput=True).ap()
    t["k_cc"] = nc.dram_tensor("k_cc", [P, NL, 2], f16).ap()
    t["k_all"] = nc.dram_tensor("k_all", [NCORES * P, NL, 2], f16,
                                addr_space="Shared").ap()

    with tile.TileContext(nc) as tc:
        build_body(tc, t)
    nc.compile()
    return nc


def make_in_maps(input_tensor, sp_matrix, W1, b1, Wq, Wk, Wv):
    x = np.asarray(input_tensor, dtype=np.float32)
    sp = np.asarray(sp_matrix).astype(np.int64)

    def wlayout(w, scale=1.0):
        w = np.asarray(w, np.float32) * scale
        # [HID, DK] -> [p, hc, e, m] = w[hc*128+p, e*128+m]
        return np.ascontiguousarray(
            w.reshape(HC, P, 2, P).transpose(1, 0, 2, 3)).astype(np.float16)

    w1 = np.ascontiguousarray(
        np.asarray(W1, np.float32).reshape(FC, P, HID).transpose(1, 0, 2)
    ).astype(np.float16)
    wq = wlayout(Wq, 1.0 / np.sqrt(np.float32(DK)))
    wk = wlayout(Wk)
    wv = wlayout(Wv)
    b1v = np.ascontiguousarray(np.asarray(b1, np.float32).reshape(HC, P).T)

    in_maps = []
    for c in range(NCORES):
        xc = x[c * NL:(c + 1) * NL]
        xt = np.ascontiguousarray(xc.T.reshape(FC, P, NL).transpose(1, 0, 2))
        spc = sp[c * NL:(c + 1) * NL]                 # [NL, 9]
        gidx = spc.T.reshape(-1).astype(np.int16)     # t = j*NL + i
        wrapped = np.zeros((16, NI // 16), np.int16)
        wrapped[:, :] = gidx.reshape(NI // 16, 16).T
        idx = np.ascontiguousarray(np.tile(wrapped, (8, 1)))
        in_maps.append({
            "xt": xt, "w1": w1, "wq": wq, "wk": wk, "wv": wv,
            "b1v": b1v, "idx": idx,
        })
    return in_maps


def combine_outputs(results, bv, Wout, bout):
    P_kk = np.zeros((3, DK), np.float64)
    Z_kk = np.zeros(3, np.float64)
    for r in results:
        op = np.asarray(r["out_p"], np.float64).reshape(P, 2, 3)   # [p, e, kk]
        for kk in range(3):
            P_kk[kk] += op[:, :, kk].T.reshape(DK)    # d = e*128+p
        Z_kk += np.asarray(r["out_z"], np.float64).reshape(3)
    pooled = (P_kk / Z_kk[:, None]).sum(axis=0) + 3.0 * np.asarray(bv, np.float64)
    logits = pooled @ np.asarray(Wout, np.float64) + np.asarray(bout, np.float64)
    logits -= logits.max()
    e = np.exp(logits)
    return (e / e.sum()).reshape(1, 2).astype(np.float32)


_NC_CACHE = {}


def kernel(input_tensor, sp_matrix, W1, b1, Wq, Wk, Wv, bv, Wout, bout):
    from concourse.bass_utils import run_bass_kernel_spmd

    if "nc" not in _NC_CACHE:
        _NC_CACHE["nc"] = build_nc()
    nc = _NC_CACHE["nc"]

    in_maps = make_in_maps(input_tensor, sp_matrix, W1, b1, Wq, Wk, Wv)
    res = run_bass_kernel_spmd(nc, in_maps, core_ids=list(range(NCORES)))
    return combine_outputs(res.results, bv, Wout, bout)


# revision 20
# speedup vs baseline: 196.1043x; 196.1043x over previous
"""Trainium2 Bass kernel for nn_CHARM_89146341196444 (gnn_message_passing).

Reference (N=8192, FEAT=1024, HID=512, DK=256, KMAX=8):
    dense = relu(X @ W1 + b1); q = dense @ Wq / 16; k = dense @ Wk
    v = dense @ Wv + bv
    a_kk[i] = sum_{j<=kk} q[i] . k[sp[i, j]]          (kk in {2,4,8})
    alpha_kk = softmax(a_kk) over all N rows
    out = softmax((sum_kk sum_i alpha_kk[i] v[i]) @ Wout + bout)   # [1, 2]

The [N, N] attention matrix is never materialized: only <=9 entries per row
are used, and each is a q.k dot — so we gather the needed k rows (GPSIMD
ap_gather from an SBUF table, no DMA descriptors) and reduce.

Distribution: rows sharded 1024/core on 8 cores. k is computed in a packed
transposed layout kT2[p, i, e] = k[i, e*128+p] (f16), AllGather'ed (4 MB),
and gathered per (row, neighbor). Per-core partial sums P_kk (=sum e_i v_i)
and Z_kk (=sum e_i) with a constant exp-shift (cancels in P/Z) return to the
host, which finishes the tiny [256] -> [1, 2] head.
"""

import numpy as np

P = 128
NL = 1024          # rows per core
FEAT = 1024
HID = 512
DK = 256
KMAX1 = 9
NCORES = 8
N = NL * NCORES
IC = NL // P       # 8
FC = FEAT // P     # 8
HC = HID // P      # 4
NI = NL * KMAX1    # 9216 gather indices per core
EXP_SHIFT = 12.0
IT = 2             # fc1 i-tiles (512 wide)
ITW = NL // IT


def build_body(tc, t):
    import concourse.mybir as mybir

    nc = tc.nc
    f16 = mybir.dt.float16
    f32 = mybir.dt.float32
    Relu = mybir.ActivationFunctionType.Relu
    Exp = mybir.ActivationFunctionType.Exp
    X = mybir.AxisListType.X
    mult = mybir.AluOpType.mult
    add = mybir.AluOpType.add
    bypass = mybir.AluOpType.bypass

    with (
        tc.tile_pool(name="wp", bufs=1) as wp,
        tc.tile_pool(name="per", bufs=1) as per,
        tc.tile_pool(name="pp", bufs=2, space="PSUM") as pp,
    ):
        # ---- weights / constants ----
        w1_sb = wp.tile([P, FC, HID], f16)
        nc.sync.dma_start(out=w1_sb[:], in_=t["w1"])
        wq_sb = wp.tile([P, HC, 2, P], f16)
        nc.sync.dma_start(out=wq_sb[:], in_=t["wq"])
        wk_sb = wp.tile([P, HC, 2, P], f16)
        nc.sync.dma_start(out=wk_sb[:], in_=t["wk"])
        wv_sb = wp.tile([P, HC, 2, P], f16)
        nc.sync.dma_start(out=wv_sb[:], in_=t["wv"])
        b1_sb = wp.tile([P, HC], f32)
        nc.sync.dma_start(out=b1_sb[:], in_=t["b1v"])
        idx_sb = wp.tile([P, NI // 16], mybir.dt.int16)
        nc.sync.dma_start(out=idx_sb[:], in_=t["idx"])
        ones_sb = wp.tile([P, P], f16)
        nc.vector.memset(ones_sb[:], 1.0)
        shift_sb = wp.tile([P, 1], f32)
        nc.vector.memset(shift_sb[:], -EXP_SHIFT)

        # ---- persistent activations ----
        denseT = per.tile([P, HC, NL], f16)       # [h, i]
        q2 = per.tile([P, NL * 2], f16)           # q[i, e*128+p], (i e) flat
        k2 = per.tile([P, NL, 2], f16)            # k interleaved
        v_cm = per.tile([P, 2, NL], f16)          # v chunk-major
        s2 = per.tile([P, NL * 2], f16)
        e_bc = per.tile([P, 3, NL], f16)          # exp(a - shift), bcast rows
        p_sb = per.tile([P, 2, 4], f32)
        z_sb = per.tile([P, 4], f32)

        # ---- fc1: denseT = relu(W1^T X^T + b1) ----
        with tc.tile_pool(name="xf", bufs=2) as xf:
            for it in range(IT):
                sl = slice(it * ITW, (it + 1) * ITW)
                xt_f = xf.tile([P, FC, ITW], f32, tag="xtf")
                nc.sync.dma_start(out=xt_f[:], in_=t["xt"][it])
                xtb = xf.tile([P, FC, ITW], f16, tag="xtb")
                nc.scalar.copy(out=xtb[:], in_=xt_f[:])
                for hc in range(HC):
                    ps = pp.tile([P, ITW], f32, tag="ps")
                    for fc in range(FC):
                        nc.tensor.matmul(
                            out=ps[:],
                            lhsT=w1_sb[:, fc, hc * P:(hc + 1) * P],
                            rhs=xtb[:, fc, :],
                            start=(fc == 0),
                            stop=(fc == FC - 1),
                        )
                    nc.scalar.activation(
                        out=denseT[:, hc, sl], in_=ps[:],
                        func=Relu, bias=b1_sb[:, hc:hc + 1],
                    )

            # ---- kT/qT/vT chunks: lhsT = W*[:, hc, e, :], rhs = denseT ----
            def proj(w_sb, writeback):
                for e in range(2):
                    for it in range(IT):
                        sl = slice(it * ITW, (it + 1) * ITW)
                        ps = pp.tile([P, ITW], f32, tag="ps")
                        for hc in range(HC):
                            nc.tensor.matmul(
                                out=ps[:],
                                lhsT=w_sb[:, hc, e, :],
                                rhs=denseT[:, hc, sl],
                                start=(hc == 0),
                                stop=(hc == HC - 1),
                            )
                        writeback(ps, e, sl)

            # k first: it feeds the AllGather
            proj(wk_sb, lambda ps, e, sl: nc.scalar.copy(
                out=k2[:, sl, e], in_=ps[:]))
            nc.sync.dma_start(out=t["k_cc"], in_=k2[:])
            if t.get("_single_core"):
                # timing stand-in for the AllGather's local traffic
                for c in range(NCORES):
                    nc.sync.dma_start(
                        out=t["k_all"][c * P:(c + 1) * P], in_=t["k_cc"])
            else:
                nc.gpsimd.collective_compute(
                    "AllGather",
                    bypass,
                    replica_groups=[list(range(NCORES))],
                    ins=[t["k_cc"].opt()],
                    outs=[t["k_all"].opt()],
                )
            q2v = q2.rearrange("p (i e) -> p i e", e=2)
            proj(wq_sb, lambda ps, e, sl: nc.scalar.copy(
                out=q2v[:, sl, e], in_=ps[:]))
            proj(wv_sb, lambda ps, e, sl: nc.scalar.copy(
                out=v_cm[:, e, sl], in_=ps[:]))

        # ---- gather table + ap_gather ----
        with (
            tc.tile_pool(name="gp", bufs=1) as gp,
            tc.tile_pool(name="sp", bufs=2) as sp,
        ):
            tab = gp.tile([P, NCORES, NL, 2], f16)
            nc.sync.dma_start(
                out=tab[:],
                in_=t["k_all"].rearrange("(c p) i e -> p c i e", p=P),
            )
            # gather f16 pairs as single f32 words (halves the cost + moves)
            tab_f32 = tab.rearrange("p c i e -> p (c i e)").bitcast(f32)

            def gather_group(j0, nj, tag):
                gt = gp.tile([P, nj * NL], f32, tag=tag)
                nc.gpsimd.ap_gather(
                    out_ap=gt[:],
                    in_ap=tab_f32[:],
                    idxs_ap=idx_sb[:, j0 * NL // 16:(j0 + nj) * NL // 16],
                    channels=P, num_elems=N, d=1, num_idxs=nj * NL,
                )
                return gt.rearrange("p (j i) -> p j i", j=nj).bitcast(f16)

            def q_dot(s, tag):
                """at[i] = sum_e (q*s)[i, e] as f16 rows for the ones-matmul."""
                aq = sp.tile([P, NL * 2], f16, tag="aq")
                nc.vector.tensor_tensor(out=aq[:], in0=q2[:], in1=s[:], op=mult)
                at = sp.tile([P, NL], f16, tag=tag)
                with nc.allow_low_precision(
                        reason="2-way f16 add; PE accumulates the 256-dot in f32"):
                    nc.vector.reduce_sum(
                        out=at[:], in_=aq.rearrange("p (i e) -> p i e", e=2)[:],
                        axis=X)
                return at

            def exp_kk(kk, ats):
                """e_bc[:, kk, :] = exp(sum(ats) - shift), via PSUM accumulation."""
                for half in range(2):
                    hs = slice(half * ITW, (half + 1) * ITW)
                    psa = pp.tile([P, ITW], f32, tag="psa")
                    for gi, at in enumerate(ats):
                        nc.tensor.matmul(
                            out=psa[:], lhsT=ones_sb[:], rhs=at[:, hs],
                            start=(gi == 0), stop=(gi == len(ats) - 1),
                        )
                    nc.scalar.activation(
                        out=e_bc[:, kk, hs], in_=psa[:],
                        func=Exp, bias=shift_sb[:],
                    )

            def pool_kk(kk):
                ev = sp.tile([P, 2, NL], f16, tag="ev")
                nc.vector.tensor_tensor(
                    out=ev[:],
                    in0=v_cm[:],
                    in1=e_bc[:, kk:kk + 1, :].to_broadcast((P, 2, NL)),
                    op=mult,
                )
                nc.vector.reduce_sum(out=p_sb[:, :, kk], in_=ev[:], axis=X)
                nc.vector.reduce_sum(out=z_sb[:, kk:kk + 1], in_=e_bc[:, kk, :],
                                     axis=X)

            gv = gather_group(0, KMAX1, "g0")

            nc.vector.tensor_tensor(out=s2[:], in0=gv[:, 0], in1=gv[:, 1], op=add)
            nc.vector.tensor_tensor(out=s2[:], in0=s2[:], in1=gv[:, 2], op=add)
            at2 = q_dot(s2, "at2")
            exp_kk(0, [at2])
            pool_kk(0)
            d4 = sp.tile([P, NL * 2], f16, tag="d4")
            nc.vector.tensor_tensor(out=d4[:], in0=gv[:, 3], in1=gv[:, 4], op=add)
            at4 = q_dot(d4, "at4")
            exp_kk(1, [at2, at4])
            pool_kk(1)
            d8 = sp.tile([P, NL * 2], f16, tag="d8")
            nc.vector.tensor_tensor(out=d8[:], in0=gv[:, 5], in1=gv[:, 6], op=add)
            nc.vector.tensor_tensor(out=d8[:], in0=d8[:], in1=gv[:, 7], op=add)
            nc.vector.tensor_tensor(out=d8[:], in0=d8[:], in1=gv[:, 8], op=add)
            at8 = q_dot(d8, "at8")
            exp_kk(2, [at2, at4, at8])
            pool_kk(2)

            nc.sync.dma_start(out=t["out_p"], in_=p_sb[:, :, :3])
            nc.sync.dma_start(out=t["out_z"], in_=z_sb[:1, :3])


def build_nc():
    import concourse.mybir as mybir
    import concourse.tile as tile
    from concourse import bacc

    f16 = mybir.dt.float16
    f32 = mybir.dt.float32

    nc = bacc.Bacc("TRN2", target_bir_lowering=False, debug=False,
                   num_devices=NCORES)
    t = {}
    t["xt"] = nc.declare_dram_parameter("xt", [IT, P, FC, ITW], f32, isOutput=False).ap()
    t["w1"] = nc.declare_dram_parameter("w1", [P, FC, HID], f16, isOutput=False).ap()
    t["wq"] = nc.declare_dram_parameter("wq", [P, HC, 2, P], f16, isOutput=False).ap()
    t["wk"] = nc.declare_dram_parameter("wk", [P, HC, 2, P], f16, isOutput=False).ap()
    t["wv"] = nc.declare_dram_parameter("wv", [P, HC, 2, P], f16, isOutput=False).ap()
    t["b1v"] = nc.declare_dram_parameter("b1v", [P, HC], f32, isOutput=False).ap()
    t["idx"] = nc.declare_dram_parameter(
        "idx", [P, NI // 16], mybir.dt.int16, isOutput=False).ap()
    t["out_p"] = nc.declare_dram_parameter("out_p", [P, 2, 3], f32, isOutput=True).ap()
    t["out_z"] = nc.declare_dram_parameter("out_z", [1, 3], f32, isOutput=True).ap()
    t["k_cc"] = nc.dram_tensor("k_cc", [P, NL, 2], f16).ap()
    t["k_all"] = nc.dram_tensor("k_all", [NCORES * P, NL, 2], f16,
                                addr_space="Shared").ap()

    with tile.TileContext(nc) as tc:
        build_body(tc, t)
    nc.compile()
    return nc


def make_in_maps(input_tensor, sp_matrix, W1, b1, Wq, Wk, Wv):
    x = np.asarray(input_tensor, dtype=np.float32)
    sp = np.asarray(sp_matrix).astype(np.int64)

    def wlayout(w, scale=1.0):
        w = np.asarray(w, np.float32) * scale
        # [HID, DK] -> [p, hc, e, m] = w[hc*128+p, e*128+m]
        return np.ascontiguousarray(
            w.reshape(HC, P, 2, P).transpose(1, 0, 2, 3)).astype(np.float16)

    w1 = np.ascontiguousarray(
        np.asarray(W1, np.float32).reshape(FC, P, HID).transpose(1, 0, 2)
    ).astype(np.float16)
    wq = wlayout(Wq, 1.0 / np.sqrt(np.float32(DK)))
    wk = wlayout(Wk)
    wv = wlayout(Wv)
    b1v = np.ascontiguousarray(np.asarray(b1, np.float32).reshape(HC, P).T)

    in_maps = []
    for c in range(NCORES):
        xc = x[c * NL:(c + 1) * NL]
        xt0 = xc.T.reshape(FC, P, NL).transpose(1, 0, 2)      # [p, fc, i]
        xt = np.ascontiguousarray(
            xt0.reshape(P, FC, IT, ITW).transpose(2, 0, 1, 3))  # [it, p, fc, w]
        spc = sp[c * NL:(c + 1) * NL]                 # [NL, 9]
        gidx = spc.T.reshape(-1).astype(np.int16)     # t = j*NL + i
        wrapped = np.zeros((16, NI // 16), np.int16)
        wrapped[:, :] = gidx.reshape(NI // 16, 16).T
        idx = np.ascontiguousarray(np.tile(wrapped, (8, 1)))
        in_maps.append({
            "xt": xt, "w1": w1, "wq": wq, "wk": wk, "wv": wv,
            "b1v": b1v, "idx": idx,
        })
    return in_maps


def combine_outputs(results, bv, Wout, bout):
    P_kk = np.zeros((3, DK), np.float64)
    Z_kk = np.zeros(3, np.float64)
    for r in results:
        op = np.asarray(r["out_p"], np.float64).reshape(P, 2, 3)   # [p, e, kk]
        for kk in range(3):
            P_kk[kk] += op[:, :, kk].T.reshape(DK)    # d = e*128+p
        Z_kk += np.asarray(r["out_z"], np.float64).reshape(3)
    pooled = (P_kk / Z_kk[:, None]).sum(axis=0) + 3.0 * np.asarray(bv, np.float64)
    logits = pooled @ np.asarray(Wout, np.float64) + np.asarray(bout, np.float64)
    logits -= logits.max()
    e = np.exp(logits)
    return (e / e.sum()).reshape(1, 2).astype(np.float32)


_NC_CACHE = {}


def kernel(input_tensor, sp_matrix, W1, b1, Wq, Wk, Wv, bv, Wout, bout):
    from concourse.bass_utils import run_bass_kernel_spmd

    if "nc" not in _NC_CACHE:
        _NC_CACHE["nc"] = build_nc()
    nc = _NC_CACHE["nc"]

    in_maps = make_in_maps(input_tensor, sp_matrix, W1, b1, Wq, Wk, Wv)
    res = run_bass_kernel_spmd(nc, in_maps, core_ids=list(range(NCORES)))
    return combine_outputs(res.results, bv, Wout, bout)
